# revision 1
# baseline (speedup 1.0000x reference)
"""Chamfer-with-normals loss kernel for Trainium2 (Bass/Tile), 8 NeuronCores.

Math (per batch item, N=4096 points):
    d[i,j] = ||ap_i - bp_j||^2 + w*(1 - <bn_i, an_j>)
           = aa[i] + bb[j] - 2<ap_i,bp_j> - w<bn_i,an_j> + w
    loss   = (sum_b [ sum_i min_j d + sum_j min_i d ]) / B

Sharding: data-parallel over batch B=8, one batch item per core. Each core
computes its 4096x4096 distance matrix tile-by-tile fully on-chip (PSUM),
reduces to a scalar partial; host sums the 8 partials.

The whole d matrix (minus the constant +w, added on host) is produced by a
single K=128 fp32 matmul per tile, with the 4 feature groups at partitions
{0:3, 32:35, 64:67, 96:99} and exact-zero rows elsewhere (K is free on the
PE; aligned starts satisfy the compute-engine SBUF partition rule):
    rows 0:3   sqrt(2)*a_pts x -sqrt(2)*b_pts -> -2<ap_i,bp_j>
    rows 32:35 sqrt(w)*b_nrm x -sqrt(w)*a_nrm -> -w<bn_i,an_j>
    rows 64:67 a_pts^2       x  1             -> aa[i]
    rows 96:99 1             x  b_pts^2       -> bb[j]

Per (row-tile, col-group) of [128, 2048] (4 PSUM banks, double-buffered):
    PE:  4 fp32 matmuls (N=512 each)
    DVE: tensor_reduce min (row-min) + running tensor_tensor min (col-min),
         both streaming straight from PSUM. The DVE is the saturated engine:
         every d element passes it twice at 1 elem/lane/cycle (~273us/core
         floor); GPSIMD/ACT cannot help (walrus rejects min on Pool, ACT has
         no min, and the TENSOR_TENSOR_REDUCE opcode faults at runtime).
Final: partition-axis min of the col-min accumulator via PE transpose + DVE
reduce; row-mins pair-reduced + summed; partition sum via one more PE
transpose; scalar DMA'd out. Host sums the 8 per-core partials.
"""

import numpy as np

import concourse.bacc as bacc
import concourse.bass as bass
import concourse.tile as tile
from concourse import mybir
from concourse.masks import make_identity

B = 8
C = 6
N = 4096
W = 0.001
P = 128
BIG = 1.0e30  # +inf surrogate (keeps finiteness checks happy)

F32 = mybir.dt.float32
F16 = mybir.dt.float16
F16BIG = 60000.0  # fp16-finite +inf surrogate; d values are O(100)
MIN = mybir.AluOpType.min
ADD = mybir.AluOpType.add
MULT = mybir.AluOpType.mult


def build_nc(n=N, g_cols=2048, colmin16=False, mm_f32r=False, repeat=1):
    """Build the single-core Bass program (SPMD across 8 cores).

    repeat>1 re-runs the (idempotent) main loop that many times inside a
    device-side For_i — used to measure true HW kernel time by wallclock
    differencing across the axon tunnel.
    """
    assert n % P == 0 and g_cols % 512 == 0 and n % g_cols == 0
    n_mt = n // P          # row tiles
    n_g = n // g_cols      # column groups

    nc = bacc.Bacc(trn_type="TRN2", debug=False, enable_partition_id=False)
    a_dram = nc.dram_tensor("a_local", [C, n], F32, kind="ExternalInput").ap()
    b_dram = nc.dram_tensor("b_local", [C, n], F32, kind="ExternalInput").ap()
    out_dram = nc.dram_tensor("out", [1, 1], F32, kind="ExternalOutput").ap()
    # Never written -> the runtime's zero-initialized output buffer doubles as
    # a zero source, so the dead-row zero-fill is DMA work instead of ~17us of
    # DVE memsets.
    zdt = mybir.dt.float32r if mm_f32r else F32
    zeros_dram = nc.dram_tensor("zeros", [32, n], zdt, kind="ExternalOutput").ap()

    with tile.TileContext(nc) as tc:
        with (
            tc.tile_pool(name="singles", bufs=1) as singles,
            tc.tile_pool(name="dtiles", bufs=3) as dpool,
        ):
            # ---------------- setup ----------------
            # K=128 matmul with zero rows: feature groups live at partitions
            # {0:3, 32:35, 64:67, 96:99} (compute-legal SBUF starts), all
            # other rows are exact zeros. Matmul cost is K-independent, so
            # this costs nothing on the PE and avoids any partition-odd
            # assembly DMAs (whose many semaphores overflow the per-
            # instruction HW wait-command limit).
            #   rows 0:3   sqrt(2)*a_pts  x  -sqrt(2)*b_pts  -> -2<ap,bp>
            #   rows 32:35 sqrt(w)*b_nrm  x  -sqrt(w)*a_nrm  -> -w<bn,an>
            #   rows 64:67 a_pts^2        x  1               -> aa[i]
            #   rows 96:99 1              x  b_pts^2         -> bb[j]
            # Scales split as +/-sqrt so every live row's LAST writer is a
            # compute engine: xt deps = {DVE}, yt deps = {GPSIMD} only.
            s2 = float(np.sqrt(2.0))
            sw = float(np.sqrt(W))
            # With mm_f32r the operand tiles are float32r-typed: every
            # matmul-visible producer must emit f32r (BIR verifier rule).
            # Compute writers round on write; raw-bit DMA writes go through
            # a bitcast view (f32 zeros/input bits are valid f32r bits).
            op_dt = mybir.dt.float32r if mm_f32r else F32
            xt = singles.tile([P, n], op_dt)  # lhsT rows (a-side features)
            yt = singles.tile([P, n], op_dt)  # rhs rows  (b-side features)

            # zero everything first (dead rows must be exact 0; live rows get
            # overwritten by the DMAs + fills below, WAW deps keep the order)
            for t in (xt, yt):
                for p0 in (0, 32, 64, 96):
                    nc.sync.dma_start(out=t[p0:p0 + 32, :], in_=zeros_dram[:, :])

            # inputs land in f32 staging tiles (same partitions as their
            # destination rows); the compute fills below write the f32r
            # operand tiles, so every matmul-visible producer is f32r.
            stage_a = singles.tile([P, n], F32)
            stage_b = singles.tile([P, n], F32)
            nc.sync.dma_start(out=stage_a[0:3, :], in_=a_dram[0:3, :])
            nc.sync.dma_start(out=stage_a[32:35, :], in_=b_dram[3:6, :])
            nc.sync.dma_start(out=stage_a[64:67, :], in_=a_dram[0:3, :])
            nc.sync.dma_start(out=stage_b[0:3, :], in_=b_dram[0:3, :])
            nc.sync.dma_start(out=stage_b[32:35, :], in_=a_dram[3:6, :])
            nc.sync.dma_start(out=stage_b[96:99, :], in_=b_dram[0:3, :])
            if mm_f32r:
                # memset can't write f32r; build the ones rows as 0+1 from
                # zeroed staging via tensor_scalar_add (a valid f32r producer)
                zv = zeros_dram[0:3, :].bitcast(F32)
                nc.sync.dma_start(out=stage_a[96:99, :], in_=zv)
                nc.sync.dma_start(out=stage_b[64:67, :], in_=zv)

            # xt transforms on DVE, yt on GPSIMD, except the yt square
            # (2-input ops are ~2x slower on GPSIMD) which goes to DVE
            nc.vector.tensor_scalar(
                out=xt[0:3, :], in0=stage_a[0:3, :], scalar1=s2, scalar2=None, op0=MULT)
            nc.vector.tensor_scalar(
                out=xt[32:35, :], in0=stage_a[32:35, :], scalar1=sw, scalar2=None, op0=MULT)
            nc.vector.tensor_tensor(
                out=xt[64:67, :], in0=stage_a[64:67, :], in1=stage_a[64:67, :], op=MULT)
            if mm_f32r:
                nc.vector.tensor_scalar(
                    out=xt[96:99, :], in0=stage_a[96:99, :],
                    scalar1=1.0, scalar2=None, op0=ADD)
            else:
                nc.gpsimd.memset(xt[96:99, :], 1.0)
            nc.gpsimd.tensor_scalar(
                out=yt[0:3, :], in0=stage_b[0:3, :], scalar1=-s2, scalar2=None, op0=MULT)
            nc.gpsimd.tensor_scalar(
                out=yt[32:35, :], in0=stage_b[32:35, :], scalar1=-sw, scalar2=None, op0=MULT)
            nc.vector.tensor_tensor(
                out=yt[96:99, :], in0=stage_b[96:99, :], in1=stage_b[96:99, :], op=MULT)
            if mm_f32r:
                nc.gpsimd.tensor_scalar(
                    out=yt[64:67, :], in0=stage_b[64:67, :],
                    scalar1=1.0, scalar2=None, op0=ADD)
            else:
                nc.gpsimd.memset(yt[64:67, :], 1.0)

            rowmins = singles.tile([P, n_mt, n_g], F32)  # per (row-tile, group) row-min
            # col-min runs in fp16: DVE tensor_tensor gets 2x mode on 16-bit
            # SBUF operands, halving the col-min pass cost. Row-min stays
            # exact fp32 via the TTR accumulator. (fp16 quantization of the
            # colmin terms costs ~3e-6 rel on the final loss.)
            cm_dt = F16 if colmin16 else F32
            colmin = singles.tile([P, n_g, g_cols], cm_dt)
            nc.vector.memset(colmin, F16BIG if colmin16 else BIG)

            # ---------------- main loop ----------------
            import contextlib
            rep_ctx = tc.For_i(0, repeat, 1) if repeat > 1 else contextlib.nullcontext()
            with tc.tile_pool(name="psum_d", bufs=2, space="PSUM") as pd_pool, rep_ctx:
                F32R = mybir.dt.float32r
                for mt in range(n_mt):
                    lhsT = xt[:, mt * P:(mt + 1) * P]
                    for g in range(n_g):
                        ps = pd_pool.tile([P, g_cols], F32, tag="ps")
                        for q in range(g_cols // 512):
                            j0 = g * g_cols + q * 512
                            rhs = yt[:, j0:j0 + 512]
                            nc.tensor.matmul(
                                ps[:, q * 512:(q + 1) * 512],
                                lhsT,
                                rhs,
                                start=True, stop=True,
                            )
                        # exact fp32 row-min straight from PSUM (DVE)
                        nc.vector.tensor_reduce(
                            out=rowmins[:, mt, g:g + 1], in_=ps,
                            axis=mybir.AxisListType.X, op=MIN,
                        )
                        if colmin16:
                            # otherwise-idle ACT engine copies d to fp16...
                            dt16 = dpool.tile([P, g_cols], F16, tag="dt16")
                            nc.scalar.activation(
                                out=dt16, in_=ps,
                                func=mybir.ActivationFunctionType.Copy,
                            )
                            # ...so the col-min runs on fp16 SBUF (DVE 2x mode)
                            nc.vector.tensor_tensor(
                                out=colmin[:, g, :], in0=dt16,
                                in1=colmin[:, g, :], op=MIN,
                            )
                        else:
                            nc.vector.tensor_tensor(
                                out=colmin[:, g, :], in0=ps,
                                in1=colmin[:, g, :], op=MIN,
                            )

            # ---------------- final reduction ----------------
            identity = singles.tile([P, P], F32)
            make_identity(nc, identity)
            identity16 = singles.tile([P, P], F16)
            make_identity(nc, identity16)

            # row side: min over groups, then sum
            rm_mt = singles.tile([P, n_mt], F32)
            nc.vector.tensor_reduce(out=rm_mt, in_=rowmins, axis=mybir.AxisListType.X, op=MIN)
            row_sum = singles.tile([P, 1], F32)
            nc.vector.tensor_reduce(out=row_sum, in_=rm_mt, axis=mybir.AxisListType.X, op=ADD)

            # col side: partition-axis min via PE transpose, 128 cols at a time
            n_chunks = n // P
            collector = singles.tile([P, n_chunks], F32)
            if colmin16:
                # convert col-min accumulator to fp32 before the PE-transpose
                # tail (16-bit transpose into PSUM is the exotic path; avoid)
                colmin32 = singles.tile([P, n_g, g_cols], F32)
                nc.vector.tensor_copy(
                    colmin32[:].rearrange("p a b -> p (a b)"),
                    colmin[:].rearrange("p a b -> p (a b)"))
                cm_src = colmin32
            else:
                cm_src = colmin
            with tc.tile_pool(name="psum_t", bufs=4, space="PSUM") as pt_pool:
                cm_flat = cm_src[:].rearrange("p a b -> p (a b)")
                for t in range(n_chunks):
                    psT = pt_pool.tile([P, P], F32, tag="psT")
                    nc.tensor.transpose(psT, cm_flat[:, t * P:(t + 1) * P], identity)
                    nc.vector.tensor_reduce(
                        out=collector[:, t:t + 1], in_=psT,
                        axis=mybir.AxisListType.X, op=MIN,
                    )

                col_sum = singles.tile([P, 1], F32)
                nc.vector.tensor_reduce(out=col_sum, in_=collector, axis=mybir.AxisListType.X, op=ADD)

                total_p = singles.tile([P, 1], F32)
                nc.vector.tensor_tensor(out=total_p, in0=row_sum, in1=col_sum, op=ADD)

                psF = pt_pool.tile([1, P], F32, tag="psF")
                nc.tensor.transpose(psF, total_p, identity)
                loss_sb = singles.tile([1, 1], F32)
                nc.vector.tensor_reduce(out=loss_sb, in_=psF, axis=mybir.AxisListType.X, op=ADD)

            nc.sync.dma_start(out=out_dram[:, :], in_=loss_sb[0:1, 0:1])

    nc.compile()  # bacc passes: split multi-waits (TRN2: 1 wait/instruction), etc.
    return nc


_NC_CACHE = {}


def _get_nc():
    if "nc" not in _NC_CACHE:
        _NC_CACHE["nc"] = build_nc()
    return _NC_CACHE["nc"]


def kernel(a: np.ndarray, b: np.ndarray) -> np.ndarray:
    """Full inputs a, b: [B, 6, N] float32 -> scalar float32 loss."""
    from concourse.bass_utils import run_bass_kernel_spmd

    a = np.ascontiguousarray(np.asarray(a), dtype=np.float32)
    b = np.ascontiguousarray(np.asarray(b), dtype=np.float32)
    assert a.shape == (B, C, N) and b.shape == (B, C, N)

    nc = _get_nc()
    in_maps = [{"a_local": a[c], "b_local": b[c]} for c in range(B)]
    res = run_bass_kernel_spmd(nc, in_maps, core_ids=list(range(B)))
    partials = [float(r["out"][0, 0]) for r in res.results]
    # each core's partial omits the +w constant inside d: min_j(core+w) = w + min_j(core),
    # contributing 2*N*w per batch item; /B at the end.
    total = (sum(partials)) / B + 2 * N * W
    return np.asarray(total, dtype=np.float32)



# revision 2
# speedup vs baseline: 1.2608x; 1.2608x over previous
"""Chamfer-with-normals loss kernel for Trainium2 (Bass/Tile), 8 NeuronCores.

Math (per batch item, N=4096 points):
    d[i,j] = ||ap_i - bp_j||^2 + w*(1 - <bn_i, an_j>)
           = aa[i] + bb[j] - 2<ap_i,bp_j> - w<bn_i,an_j> + w
    loss   = (sum_b [ sum_i min_j d + sum_j min_i d ]) / B

Sharding: data-parallel over batch B=8, one batch item per core. Each core
computes its 4096x4096 distance matrix tile-by-tile fully on-chip (PSUM),
reduces to a scalar partial; host sums the 8 partials.

The whole d matrix (minus the constant +w, added on host) is produced by
K=128 float32r matmuls (f32r streams at 1 col/cycle on the PE vs 4 for
plain fp32), with the 4 feature groups at partitions {0:3, 32:35, 64:67,
96:99} and exact-zero rows elsewhere:
    rows 0:3   sqrt(2)*a_pts x -sqrt(2)*b_pts -> -2<ap_i,bp_j>
    rows 32:35 sqrt(w)*b_nrm x -sqrt(w)*a_nrm -> -w<bn_i,an_j>
    rows 64:67 a_pts^2       x  1             -> aa[i]
    rows 96:99 1             x  b_pts^2       -> bb[j]

Engine assignment per row-tile mt (two [128,2048] PSUM tiles):
    PE:  8 f32r matmuls (N=512 each) into the two PSUM tiles
    ACT: 2 activation-Copies PSUM -> one wide fp16 SBUF tile dt16w[128,4096]
         (ACT is the only PSUM reader; ~1.9us per copy, hidden under DVE)
    DVE: 3 ops, all on 16-bit SBUF operands (2x perf mode where available):
         - col-min: tensor_tensor min of dt16w into colmin16[128,4096]
         - fold:    tensor_tensor min of dt16w halves -> tmp16[128,2048]
         - row-min: tensor_reduce min of tmp16 -> rowmins[:, mt]
    DVE is the saturated engine at ~6.7us/mt (~184us/core main loop); the
    exact-fp32 alternative (reduce+TT straight from PSUM at 1x) is ~40%
    slower, and fused reduce ops (TENSOR_TENSOR_REDUCE/TENSOR_MASK_REDUCE)
    fault at runtime. GPSIMD (walrus rejects min/max on Pool) and ACT (no
    min) cannot carry either min pass.

Final: partition-axis min of colmin16 via fp32 convert + PE transposes +
DVE reduces; row sum via one more PE transpose; scalar DMA'd out. Host
sums the 8 per-core partials. fp16 quantization + f32r matmul rounding
cost ~4e-4 relative on the final loss.
"""

import numpy as np

import concourse.bacc as bacc
import concourse.bass as bass
import concourse.tile as tile
from concourse import mybir
from concourse.masks import make_identity

B = 8
C = 6
N = 4096
W = 0.001
P = 128

F32 = mybir.dt.float32
F32R = mybir.dt.float32r
F16 = mybir.dt.float16
F16BIG = 60000.0  # fp16-finite +inf surrogate; d values are O(100)
MIN = mybir.AluOpType.min
ADD = mybir.AluOpType.add
MULT = mybir.AluOpType.mult


def build_nc(n=N, g_cols=2048, repeat=1):
    """Build the single-core Bass program (SPMD across 8 cores).

    repeat>1 re-runs the (idempotent) main loop that many times inside a
    device-side For_i — used to measure true HW kernel time by wallclock
    differencing across the axon tunnel.
    """
    assert n % P == 0 and g_cols % 512 == 0 and n % g_cols == 0
    n_mt = n // P          # row tiles
    n_g = n // g_cols      # column groups per row tile

    nc = bacc.Bacc(trn_type="TRN2", debug=False, enable_partition_id=False)
    a_dram = nc.dram_tensor("a_local", [C, n], F32, kind="ExternalInput").ap()
    b_dram = nc.dram_tensor("b_local", [C, n], F32, kind="ExternalInput").ap()
    out_dram = nc.dram_tensor("out", [1, 1], F32, kind="ExternalOutput").ap()
    # Never written -> the runtime's zero-initialized output buffer doubles as
    # a zero source, so the dead-row zero-fill is DMA work instead of ~17us of
    # DVE memsets. f32r-typed so the raw-bit DMA writes satisfy the BIR
    # verifier rule that every matmul-visible producer emits f32r.
    zeros_dram = nc.dram_tensor("zeros", [32, n], F32R, kind="ExternalOutput").ap()

    with tile.TileContext(nc) as tc:
        with (
            tc.tile_pool(name="singles", bufs=1) as singles,
        ):
            # ---------------- operand setup ----------------
            # K=128 matmul with zero rows: feature groups live at partitions
            # {0:3, 32:35, 64:67, 96:99} (compute-legal SBUF starts), all
            # other rows exact zeros. Matmul cost is K-independent, so the
            # dead rows cost nothing on the PE.
            # Scales split as +/-sqrt so every live row's LAST writer is a
            # compute engine emitting f32r (BIR verifier rule); raw-bit DMA
            # writes go through the f32r-typed zeros tensor.
            s2 = float(np.sqrt(2.0))
            sw = float(np.sqrt(W))
            xt = singles.tile([P, n], F32R)  # lhsT rows (a-side features)
            yt = singles.tile([P, n], F32R)  # rhs rows  (b-side features)

            # zero everything first (live rows overwritten below; WAW deps
            # keep the order)
            for t in (xt, yt):
                for p0 in (0, 32, 64, 96):
                    nc.sync.dma_start(out=t[p0:p0 + 32, :], in_=zeros_dram[:, :])

            # inputs land in f32 staging tiles (same partitions as their
            # destination rows); the compute fills below write the f32r
            # operand tiles, so every matmul-visible producer is f32r.
            stage_a = singles.tile([P, n], F32)
            stage_b = singles.tile([P, n], F32)
            nc.sync.dma_start(out=stage_a[0:3, :], in_=a_dram[0:3, :])
            nc.sync.dma_start(out=stage_a[32:35, :], in_=b_dram[3:6, :])
            nc.sync.dma_start(out=stage_a[64:67, :], in_=a_dram[0:3, :])
            nc.sync.dma_start(out=stage_b[0:3, :], in_=b_dram[0:3, :])
            nc.sync.dma_start(out=stage_b[32:35, :], in_=a_dram[3:6, :])
            nc.sync.dma_start(out=stage_b[96:99, :], in_=b_dram[0:3, :])
            # memset can't write f32r; build the ones rows as 0+1 from zeroed
            # staging via tensor_scalar_add (a valid f32r producer)
            zv = zeros_dram[0:3, :].bitcast(F32)
            nc.sync.dma_start(out=stage_a[96:99, :], in_=zv)
            nc.sync.dma_start(out=stage_b[64:67, :], in_=zv)

            # xt transforms on DVE, yt on GPSIMD, except the yt square
            # (2-input ops are ~2x slower on GPSIMD) which goes to DVE
            nc.vector.tensor_scalar(
                out=xt[0:3, :], in0=stage_a[0:3, :], scalar1=s2, scalar2=None, op0=MULT)
            nc.vector.tensor_scalar(
                out=xt[32:35, :], in0=stage_a[32:35, :], scalar1=sw, scalar2=None, op0=MULT)
            nc.vector.tensor_tensor(
                out=xt[64:67, :], in0=stage_a[64:67, :], in1=stage_a[64:67, :], op=MULT)
            nc.vector.tensor_scalar(
                out=xt[96:99, :], in0=stage_a[96:99, :], scalar1=1.0, scalar2=None, op0=ADD)
            nc.gpsimd.tensor_scalar(
                out=yt[0:3, :], in0=stage_b[0:3, :], scalar1=-s2, scalar2=None, op0=MULT)
            nc.gpsimd.tensor_scalar(
                out=yt[32:35, :], in0=stage_b[32:35, :], scalar1=-sw, scalar2=None, op0=MULT)
            nc.vector.tensor_tensor(
                out=yt[96:99, :], in0=stage_b[96:99, :], in1=stage_b[96:99, :], op=MULT)
            nc.gpsimd.tensor_scalar(
                out=yt[64:67, :], in0=stage_b[64:67, :], scalar1=1.0, scalar2=None, op0=ADD)

            colmin16 = singles.tile([P, n], F16)
            nc.vector.memset(colmin16, F16BIG)
            rowmins = singles.tile([P, n_mt], F32)

            # ---------------- main loop ----------------
            import contextlib
            rep_ctx = tc.For_i(0, repeat, 1) if repeat > 1 else contextlib.nullcontext()
            with (
                tc.tile_pool(name="psum_d", bufs=2, space="PSUM") as pd_pool,
                tc.tile_pool(name="dtiles", bufs=3) as dpool,
                rep_ctx,
            ):
                for mt in range(n_mt):
                    lhsT = xt[:, mt * P:(mt + 1) * P]
                    dt16w = dpool.tile([P, n], F16, tag="dt16w")
                    for g in range(n_g):
                        ps = pd_pool.tile([P, g_cols], F32, tag="ps")
                        for q in range(g_cols // 512):
                            j0 = g * g_cols + q * 512
                            nc.tensor.matmul(
                                ps[:, q * 512:(q + 1) * 512],
                                lhsT,
                                yt[:, j0:j0 + 512],
                                start=True, stop=True,
                            )
                        # otherwise-idle ACT engine moves d to fp16 SBUF and
                        # releases the PSUM bank for the PE
                        nc.scalar.activation(
                            out=dt16w[:, g * g_cols:(g + 1) * g_cols], in_=ps,
                            func=mybir.ActivationFunctionType.Copy)
                    # DVE 16-bit ops: wide col-min accumulate + fold + row-min
                    nc.vector.tensor_tensor(
                        out=colmin16, in0=dt16w, in1=colmin16, op=MIN)
                    half = n // 2
                    tmp16 = dpool.tile([P, half], F16, tag="tmp16")
                    nc.vector.tensor_tensor(
                        out=tmp16, in0=dt16w[:, 0:half], in1=dt16w[:, half:n], op=MIN)
                    nc.vector.tensor_reduce(
                        out=rowmins[:, mt:mt + 1], in_=tmp16,
                        axis=mybir.AxisListType.X, op=MIN,
                    )

            # ---------------- final reduction ----------------
            identity = singles.tile([P, P], F32)
            make_identity(nc, identity)

            row_sum = singles.tile([P, 1], F32)
            nc.vector.tensor_reduce(
                out=row_sum, in_=rowmins, axis=mybir.AxisListType.X, op=ADD)

            # col side: partition-axis min via fp32 convert + PE transposes
            # (16-bit transpose into PSUM is the exotic path; avoid)
            colmin32 = singles.tile([P, n], F32)
            nc.vector.tensor_copy(colmin32[:], colmin16[:])
            n_chunks = n // P
            collector = singles.tile([P, n_chunks], F32)
            col_sum = singles.tile([P, 1], F32)
            with tc.tile_pool(name="psum_t", bufs=4, space="PSUM") as pt_pool:
                for t in range(n_chunks):
                    psT = pt_pool.tile([P, P], F32, tag="psT")
                    nc.tensor.transpose(psT, colmin32[:, t * P:(t + 1) * P], identity)
                    nc.vector.tensor_reduce(
                        out=collector[:, t:t + 1], in_=psT,
                        axis=mybir.AxisListType.X, op=MIN,
                    )
                nc.vector.tensor_reduce(
                    out=col_sum, in_=collector, axis=mybir.AxisListType.X, op=ADD)

                total_p = singles.tile([P, 1], F32)
                nc.vector.tensor_tensor(out=total_p, in0=row_sum, in1=col_sum, op=ADD)

                psF = pt_pool.tile([1, P], F32, tag="psF")
                nc.tensor.transpose(psF, total_p, identity)
                loss_sb = singles.tile([1, 1], F32)
                nc.vector.tensor_reduce(
                    out=loss_sb, in_=psF, axis=mybir.AxisListType.X, op=ADD)

            nc.sync.dma_start(out=out_dram[:, :], in_=loss_sb[0:1, 0:1])

    nc.compile()  # bacc passes: split multi-waits (TRN2: 1 wait/instruction), etc.
    return nc


_NC_CACHE = {}


def _get_nc():
    if "nc" not in _NC_CACHE:
        _NC_CACHE["nc"] = build_nc()
    return _NC_CACHE["nc"]


def kernel(a: np.ndarray, b: np.ndarray) -> np.ndarray:
    """Full inputs a, b: [B, 6, N] float32 -> scalar float32 loss."""
    from concourse.bass_utils import run_bass_kernel_spmd

    a = np.ascontiguousarray(np.asarray(a), dtype=np.float32)
    b = np.ascontiguousarray(np.asarray(b), dtype=np.float32)
    assert a.shape == (B, C, N) and b.shape == (B, C, N)

    nc = _get_nc()
    in_maps = [{"a_local": a[c], "b_local": b[c]} for c in range(B)]
    res = run_bass_kernel_spmd(nc, in_maps, core_ids=list(range(B)))
    partials = [float(r["out"][0, 0]) for r in res.results]
    # each core's partial omits the +w constant inside d: min_j(core+w) = w + min_j(core),
    # contributing 2*N*w per batch item; /B at the end.
    total = (sum(partials)) / B + 2 * N * W
    return np.asarray(total, dtype=np.float32)
